# revision 17
# baseline (speedup 1.0000x reference)
"""Trainium2 Bass kernel for nn_FactorizedCrossAttention.

Algebraic restructure (verified exact in fp32 vs the reference):
  * spatial == temporal (cross-attention is per-row; qt rows == qs rows), so
    concat([A,A]) @ Wst @ Wo == A @ Weff with Weff = (Wst[:D]+Wst[D:]) @ Wo.
  * Q is never materialized: scores = X @ M with M_h = Wq_h @ (K_h*scale)^T
    folded on the host ([1024, 16*77] packed at stride 80 -> [1024, 1280]).
  * Weff is folded into V on the host: Vt_h = V_h @ Weff[64h:64h+64, :], so
    out = P_norm @ Vt sums over all (head, token) pairs in one GEMM.
  * softmax: S^T tiles keep tokens on partitions, so the padding mask is a
    free per-partition bias on the ACT exp.  Row sums for ALL heads land in
    one [16, 512] PSUM tile via block-indicator matmuls; one DVE reciprocal
    + selector matmuls broadcast 1/s back to token partitions.

Device layout: everything "transposed" ([feature/token part, row free]) until
the final GEMM, which uses P_norm^T as the stationary operand so the output
comes out row-major for dense DMA.

Scheduling: per 512-row tile the PE runs 180 uniform 512-cycle matmuls
(80 scores + 10 sums + 10 bcast + 80 PV*Weff of the previous row tile)
with the previous row tile's GEMM interleaved to cover the softmax
reciprocal latency, keeping the PE out of its low p-states.

Sharding: pure data-parallel over (B, T_frames): 32 frames / 8 cores.
No collectives.
"""

import sys

if "/opt/trn_rl_repo" not in sys.path:
    sys.path.insert(0, "/opt/trn_rl_repo")

from contextlib import ExitStack

import ml_dtypes
import numpy as np

import concourse.bass as bass
import concourse.mybir as mybir
import concourse.tile as tile
from concourse import bacc
from concourse.bass_utils import run_bass_kernel_spmd

BF16 = ml_dtypes.bfloat16

D = 1024           # d_model
H = 16             # num heads
G = 4              # query groups
HD = 64            # head dim
HPG = H // G
SCALE = 0.125
B, T, HW, TT = 2, 16, 1024, 77
NCORES = 8
FPC = (B * T) // NCORES      # frames per core = 4
ROWS = FPC * HW              # 4096 query rows per core
RT = 512                     # rows per row-tile
NRT = ROWS // RT             # 8
ND = D // 128                # 8 partition chunks of d_model
HS = 80                      # per-head stride in the packed token axis
NT = H * HS                  # 1280 packed (head, token) rows
NTT = NT // 128              # 10 token tiles

_PROG_CACHE = {}


def _patch_act_tables():
    """Pin every activation to the one table set containing Exp and Copy so
    bacc never emits mid-kernel ACT_TABLE_LOAD switches."""
    import concourse.bacc as _bm
    import concourse.hw_specs as _hw
    if getattr(_bm, "_act_tables_patched", False):
        return
    _orig = _hw.get_activation_tables

    def patched(arch):
        t = dict(_orig(arch))
        combo = None
        for name, funcs in t.items():
            if (mybir.ActivationFunctionType.Exp in funcs
                    and mybir.ActivationFunctionType.Ln in funcs
                    and mybir.ActivationFunctionType.Copy in funcs):
                combo = name
                break
        if combo is not None:
            for name in list(t):
                if name != combo:
                    t[name] = set()
        return t

    _bm.get_activation_tables = patched
    _bm._act_tables_patched = True

# test.py can flip these for profiling runs
TRACE = False
TRACE_KWARGS = {}
LAST_RESULTS = None


def _build_program():
    _patch_act_tables()
    dt = mybir.dt
    nc = bacc.Bacc("TRN2", target_bir_lowering=False, debug=False,
                   num_devices=NCORES)

    xt = nc.dram_tensor("xt", [D, ROWS], dt.bfloat16, kind="ExternalInput").ap()
    mt = nc.dram_tensor("mt", [D, NT], dt.bfloat16, kind="ExternalInput").ap()
    vt = nc.dram_tensor("vt", [NT, D], dt.bfloat16, kind="ExternalInput").ap()
    ind = nc.dram_tensor("ind", [128, NTT * H], dt.bfloat16, kind="ExternalInput").ap()
    sel = nc.dram_tensor("sel", [H, NTT * 128], dt.bfloat16, kind="ExternalInput").ap()
    ebias = nc.dram_tensor("ebias", [128, NTT], dt.float32, kind="ExternalInput").ap()
    out = nc.dram_tensor("out", [ROWS, D], dt.bfloat16, kind="ExternalOutput").ap()

    with tile.TileContext(nc) as tc, ExitStack() as ctx:
        wpool = ctx.enter_context(tc.tile_pool(name="weights", bufs=1))
        xpool = ctx.enter_context(tc.tile_pool(name="xt", bufs=2))
        ptpool = ctx.enter_context(tc.tile_pool(name="pt", bufs=2))
        pnpool = ctx.enter_context(tc.tile_pool(name="pn", bufs=2))
        rcpool = ctx.enter_context(tc.tile_pool(name="recip", bufs=2))
        otpool = ctx.enter_context(tc.tile_pool(name="osb", bufs=3))
        # 8 PSUM banks: scores(3) + sums(1) + bcast(2) + pvw out(2)
        spsum = ctx.enter_context(tc.tile_pool(name="spsum", bufs=3, space="PSUM"))
        supsum = ctx.enter_context(tc.tile_pool(name="supsum", bufs=1, space="PSUM"))
        rbpsum = ctx.enter_context(tc.tile_pool(name="rbpsum", bufs=2, space="PSUM"))
        opsum = ctx.enter_context(tc.tile_pool(name="opsum", bufs=2, space="PSUM"))

        # --- resident weights, loaded on the ACT hwdge queue so they overlap
        # the first xt load (sync queue).  mt is split per k-chunk so the
        # first score matmuls only wait for chunk 0.
        ebias_t = wpool.tile([128, NTT], dt.float32, tag="ebias")
        nc.scalar.dma_start(out=ebias_t[:], in_=ebias[:, :])
        # split the startup-critical loads (first xt rowtile + all M chunks,
        # ~3.5MB) across both hwdge queues in consumption order
        first_xts = []
        for kc in range(ND):
            t = xpool.tile([128, RT], dt.bfloat16, tag=f"xt{kc}")
            nc.sync.dma_start(out=t[:], in_=xt[kc * 128:(kc + 1) * 128, 0:RT])
            first_xts.append(t)
        mt_ts = []
        for kc in range(ND):
            mtc = wpool.tile([128, NT], dt.bfloat16, tag=f"mt{kc}")
            eng = nc.scalar if kc < 6 else nc.sync
            eng.dma_start(out=mtc[:], in_=mt[kc * 128:(kc + 1) * 128, :])
            mt_ts.append(mtc)
        ind_t = wpool.tile([128, NTT * H], dt.bfloat16, tag="ind")
        nc.scalar.dma_start(out=ind_t[:], in_=ind[:, :])
        sel_t = wpool.tile([H, NTT * 128], dt.bfloat16, tag="sel")
        nc.scalar.dma_start(out=sel_t[:], in_=sel[:, :])
        vt_t = wpool.tile([128, NTT * D], dt.bfloat16, tag="vt")
        for st in range(NTT):
            nc.scalar.dma_start(out=vt_t[:, st * D:(st + 1) * D],
                                in_=vt[st * 128:(st + 1) * 128, :])

        def pvw_quarter(pat, prt, rc, fillers=None):
            """One 128-row chunk of the previous rowtile's P_norm @ Vt.
            fillers: closures emitting one independent PE op each, interleaved
            every other matmul to overlap softmax bcast with this GEMM."""
            ot = otpool.tile([128, D], dt.bfloat16, tag="ot")
            for oc in range(2):
                op_ = opsum.tile([128, RT], dt.float32, tag="op")
                for st in range(NTT):
                    nc.tensor.matmul(
                        op_[:],
                        lhsT=pat[:, st * RT + rc * 128: st * RT + (rc + 1) * 128],
                        rhs=vt_t[:, st * D + oc * RT: st * D + (oc + 1) * RT],
                        start=(st == 0), stop=(st == NTT - 1),
                    )
                    if fillers and st % 2 == 1:
                        fillers.pop(0)()
                if oc == 0:
                    nc.scalar.copy(ot[:, oc * RT:(oc + 1) * RT], op_[:])
                else:
                    nc.vector.tensor_copy(ot[:, oc * RT:(oc + 1) * RT], op_[:])
                dma_eng = nc.sync if (2 * rc + oc) % 2 == 0 else nc.scalar
                dma_eng.dma_start(
                    out=out[prt * RT + rc * 128: prt * RT + (rc + 1) * 128,
                            oc * RT:(oc + 1) * RT],
                    in_=ot[:, oc * RT:(oc + 1) * RT],
                )

        def load_xt(rt):
            ts = []
            for kc in range(ND):
                t = xpool.tile([128, RT], dt.bfloat16, tag=f"xt{kc}")
                nc.sync.dma_start(
                    out=t[:],
                    in_=xt[kc * 128:(kc + 1) * 128, rt * RT:(rt + 1) * RT])
                ts.append(t)
            return ts

        def score_group(xts, pt_t, st):
            sp = spsum.tile([128, RT], dt.float32, tag="sp")
            for kc in range(ND):
                nc.tensor.matmul(
                    sp[:],
                    lhsT=mt_ts[kc][:, st * 128:(st + 1) * 128],
                    rhs=xts[kc][:],
                    start=(kc == 0), stop=(kc == ND - 1),
                )
            nc.scalar.activation(pt_t[:, st * RT:(st + 1) * RT], sp[:],
                                 mybir.ActivationFunctionType.Exp,
                                 bias=ebias_t[:, st:st + 1])

        NPULL = 4  # next-rowtile score groups pulled in to cover 1/s latency
        xts = first_xts
        pt_t = ptpool.tile([128, NTT * RT], dt.bfloat16, tag="pt")
        nxt = (xts, pt_t)
        prev = None
        for rt in range(NRT):
            xts, pt_t = nxt
            for st in range(0 if rt == 0 else NPULL, NTT):
                score_group(xts, pt_t, st)

            # --- all-head row sums -> [16, 512] PSUM
            su = supsum.tile([H, RT], dt.float32, tag="su")
            for st in range(NTT):
                nc.tensor.matmul(
                    su[:],
                    lhsT=ind_t[:, st * H:(st + 1) * H],
                    rhs=pt_t[:, st * RT:(st + 1) * RT],
                    start=(st == 0), stop=(st == NTT - 1),
                )
            # 1/s = exp(-ln s) on ACT (writes bf16 directly); the DVE
            # InstReciprocal measures ~3.3us and sits on the critical path.
            rc_f = rcpool.tile([H, RT], dt.float32, tag="rcf")
            nc.scalar.activation(rc_f[:], su[:],
                                 mybir.ActivationFunctionType.Ln)
            rc_b = rcpool.tile([H, RT], dt.bfloat16, tag="rcb")
            nc.scalar.activation(rc_b[:], rc_f[:],
                                 mybir.ActivationFunctionType.Exp,
                                 scale=-1.0)

            # cover the 1/s latency with independent PE work: the next
            # rowtile's first score groups, then PVW of the previous rowtile
            if rt + 1 < NRT:
                nxts = load_xt(rt + 1)
                npt = ptpool.tile([128, NTT * RT], dt.bfloat16, tag="pt")
                for st in range(NPULL):
                    score_group(nxts, npt, st)
                nxt = (nxts, npt)
            if prev is not None:
                pvw_quarter(prev[1], prev[0], 0)
                pvw_quarter(prev[1], prev[0], 1)

            # --- broadcast 1/s to token partitions, normalize P
            # (interleaved into the PVW GEMM so the single-matmul rb groups
            # never stall the PE on the DVE multiply)
            pn_t = pnpool.tile([128, NTT * RT], dt.bfloat16, tag="pn")

            def mk_bcast(st, pt_ref, pn_ref, rc_ref):
                def emit():
                    rb = rbpsum.tile([128, RT], dt.float32, tag="rb")
                    nc.tensor.matmul(
                        rb[:],
                        lhsT=sel_t[:, st * 128:(st + 1) * 128],
                        rhs=rc_ref[:],
                        start=True, stop=True,
                    )
                    nc.vector.tensor_mul(
                        pn_ref[:, st * RT:(st + 1) * RT],
                        pt_ref[:, st * RT:(st + 1) * RT], rb[:],
                    )
                return emit

            fillers = [mk_bcast(st, pt_t, pn_t, rc_b) for st in range(NTT)]
            if prev is not None:
                pvw_quarter(prev[1], prev[0], 2, fillers)
                pvw_quarter(prev[1], prev[0], 3)
            else:
                for f in fillers:
                    f()
            prev = (rt, pn_t)

        # drain: PVW of the final rowtile
        for rc4 in range(4):
            pvw_quarter(prev[1], prev[0], rc4)

    nc.compile()
    return nc


def _get_program():
    if "p" not in _PROG_CACHE:
        _PROG_CACHE["p"] = _build_program()
    return _PROG_CACHE["p"]


def _prep_inputs(x, te, mask, Wq, Wk, Wv, Wo, Wst):
    """Host-side fp32 weight folding + per-core shard maps."""
    Weff = ((Wst[:D] + Wst[D:]) @ Wo).astype(np.float32)

    # packed-token-axis structure: global row g -> head g//HS, token g%HS
    g = np.arange(NT)
    head_of = g // HS
    tok_of = g % HS
    real = tok_of < TT

    ind_np = np.zeros((128, NTT * H), np.float32)
    sel_np = np.zeros((H, NTT * 128), np.float32)
    for st in range(NTT):
        for p in range(128):
            gg = st * 128 + p
            if real[gg]:
                ind_np[p, st * H + head_of[gg]] = 1.0
                sel_np[head_of[gg], st * 128 + p] = 1.0

    mt_b, vt_b, eb_b = [], [], []
    for b in range(B):
        K = ((te[b] @ Wk).reshape(TT, G, HD) * SCALE).astype(np.float32)
        V = (te[b] @ Wv).reshape(TT, G, HD).astype(np.float32)
        Mp = np.zeros((D, NT), np.float32)
        Vp = np.zeros((NT, D), np.float32)
        for h in range(H):
            gq = h // HPG
            Mp[:, h * HS:h * HS + TT] = Wq[:, h * HD:(h + 1) * HD] @ K[:, gq, :].T
            Vp[h * HS:h * HS + TT] = V[:, gq, :] @ Weff[h * HD:(h + 1) * HD, :]
        mt_b.append(Mp.astype(BF16))
        vt_b.append(Vp.astype(BF16))
        # exp bias: 0 for attended tokens, -30 for masked/pad rows
        eb = np.full(NT, -30.0, np.float32)
        eb[real] = np.where(mask[b][tok_of[real]], 0.0, -30.0)
        eb_b.append(np.ascontiguousarray(eb.reshape(NTT, 128).T))

    ind_np = ind_np.astype(BF16)
    sel_np = sel_np.astype(BF16)

    in_maps = []
    for c in range(NCORES):
        b = c // (NCORES // B)
        fr = (c % (NCORES // B)) * FPC
        xc = x[b, fr:fr + FPC].reshape(ROWS, D).astype(BF16)
        in_maps.append({
            "xt": np.ascontiguousarray(xc.T),
            "mt": mt_b[b],
            "vt": vt_b[b],
            "ind": ind_np,
            "sel": sel_np,
            "ebias": eb_b[b],
        })
    return in_maps


def kernel(x, text_embeddings, padding_mask, use_mqa=0, use_qk_norm=0,
           Wq=None, Wk=None, Wv=None, Wo=None, Wst=None):
    global LAST_RESULTS
    x = np.asarray(x, np.float32)
    te = np.asarray(text_embeddings, np.float32)
    mask = np.asarray(padding_mask).astype(bool)
    Wq = np.asarray(Wq, np.float32)
    Wk = np.asarray(Wk, np.float32)
    Wv = np.asarray(Wv, np.float32)
    Wo = np.asarray(Wo, np.float32)
    Wst = np.asarray(Wst, np.float32)
    assert x.shape == (B, T, HW, D) and te.shape == (B, TT, D)

    in_maps = _prep_inputs(x, te, mask, Wq, Wk, Wv, Wo, Wst)
    nc = _get_program()

    res = run_bass_kernel_spmd(nc, in_maps, list(range(NCORES)),
                               trace=TRACE, **TRACE_KWARGS)
    LAST_RESULTS = res

    outp = np.empty((B, T, HW, D), np.float32)
    for c in range(NCORES):
        b = c // (NCORES // B)
        fr = (c % (NCORES // B)) * FPC
        outp[b, fr:fr + FPC] = res.results[c]["out"].astype(np.float32).reshape(FPC, HW, D)
    return outp


# revision 27
# speedup vs baseline: 1.0273x; 1.0273x over previous
"""Trainium2 Bass kernel for nn_FactorizedCrossAttention.

Algebraic restructure (verified exact in fp32 vs the reference):
  * spatial == temporal (cross-attention is per-row; qt rows == qs rows), so
    concat([A,A]) @ Wst @ Wo == A @ Weff with Weff = (Wst[:D]+Wst[D:]) @ Wo.
  * Q is never materialized: scores = X @ M with M_h = Wq_h @ (K_h*scale)^T
    folded on the host ([1024, 16*77] packed at stride 80 -> [1024, 1280]).
  * Weff is folded into V on the host: Vt_h = V_h @ Weff[64h:64h+64, :], so
    out = P_norm @ Vt sums over all (head, token) pairs in one GEMM.
  * softmax: S^T tiles keep tokens on partitions, so the padding mask is a
    free per-partition bias on the ACT exp.  Row sums for ALL heads land in
    one [16, 512] PSUM tile via block-indicator matmuls; one DVE reciprocal
    + selector matmuls broadcast 1/s back to token partitions.

Device layout: everything "transposed" ([feature/token part, row free]) until
the final GEMM, which uses P_norm^T as the stationary operand so the output
comes out row-major for dense DMA.

Scheduling: per 512-row tile the PE runs 180 uniform 512-cycle matmuls
(80 scores + 10 sums + 10 bcast + 80 PV*Weff of the previous row tile)
with the previous row tile's GEMM interleaved to cover the softmax
reciprocal latency, keeping the PE out of its low p-states.

Sharding: pure data-parallel over (B, T_frames): 32 frames / 8 cores.
No collectives.
"""

import sys

if "/opt/trn_rl_repo" not in sys.path:
    sys.path.insert(0, "/opt/trn_rl_repo")

from contextlib import ExitStack

import ml_dtypes
import numpy as np

import concourse.bass as bass
import concourse.mybir as mybir
import concourse.tile as tile
from concourse import bacc
from concourse.bass_utils import run_bass_kernel_spmd

BF16 = ml_dtypes.bfloat16

D = 1024           # d_model
H = 16             # num heads
G = 4              # query groups
HD = 64            # head dim
HPG = H // G
SCALE = 0.125
B, T, HW, TT = 2, 16, 1024, 77
NCORES = 8
FPC = (B * T) // NCORES      # frames per core = 4
ROWS = FPC * HW              # 4096 query rows per core
RT = 512                     # rows per row-tile
NRT = ROWS // RT             # 8
ND = D // 128                # 8 partition chunks of d_model
HS = 80                      # per-head stride in the packed token axis
NT = H * HS                  # 1280 packed (head, token) rows
NTT = NT // 128              # 10 token tiles

_PROG_CACHE = {}


def _patch_act_tables():
    """Pin every activation to the one table set containing Exp and Copy so
    bacc never emits mid-kernel ACT_TABLE_LOAD switches."""
    import concourse.bacc as _bm
    import concourse.hw_specs as _hw
    if getattr(_bm, "_act_tables_patched", False):
        return
    _orig = _hw.get_activation_tables

    def patched(arch):
        t = dict(_orig(arch))
        combo = None
        for name, funcs in t.items():
            if (mybir.ActivationFunctionType.Exp in funcs
                    and mybir.ActivationFunctionType.Ln in funcs
                    and mybir.ActivationFunctionType.Copy in funcs):
                combo = name
                break
        if combo is not None:
            for name in list(t):
                if name != combo:
                    t[name] = set()
        return t

    _bm.get_activation_tables = patched
    _bm._act_tables_patched = True

# test.py can flip these for profiling runs
TRACE = False
TRACE_KWARGS = {}
LAST_RESULTS = None


def _build_program():
    _patch_act_tables()
    dt = mybir.dt
    nc = bacc.Bacc("TRN2", target_bir_lowering=False, debug=False,
                   num_devices=NCORES)

    xt = nc.dram_tensor("xt", [D, ROWS], dt.bfloat16, kind="ExternalInput").ap()
    mt = nc.dram_tensor("mt", [D, NT], dt.bfloat16, kind="ExternalInput").ap()
    vt = nc.dram_tensor("vt", [NT, D], dt.bfloat16, kind="ExternalInput").ap()
    ind = nc.dram_tensor("ind", [128, NTT * H], dt.float8e4, kind="ExternalInput").ap()
    sel = nc.dram_tensor("sel", [H, NTT * 128], dt.bfloat16, kind="ExternalInput").ap()
    ebias = nc.dram_tensor("ebias", [128, NTT], dt.float32, kind="ExternalInput").ap()
    out = nc.dram_tensor("out", [ROWS, D], dt.bfloat16, kind="ExternalOutput").ap()

    with tile.TileContext(nc) as tc, ExitStack() as ctx:
        wpool = ctx.enter_context(tc.tile_pool(name="weights", bufs=1))
        xpool = ctx.enter_context(tc.tile_pool(name="xt", bufs=2))
        ptpool = ctx.enter_context(tc.tile_pool(name="pt", bufs=2))
        pt8pool = ctx.enter_context(tc.tile_pool(name="pt8", bufs=2))
        pnpool = ctx.enter_context(tc.tile_pool(name="pn", bufs=2))
        rcpool = ctx.enter_context(tc.tile_pool(name="recip", bufs=2))
        otpool = ctx.enter_context(tc.tile_pool(name="osb", bufs=3))
        # 8 PSUM banks: scores(2) + sums(2: one per half-group) + bcast(2) + pvw out(2)
        spsum = ctx.enter_context(tc.tile_pool(name="spsum", bufs=2, space="PSUM"))
        supsum = ctx.enter_context(tc.tile_pool(name="supsum", bufs=2, space="PSUM"))
        rbpsum = ctx.enter_context(tc.tile_pool(name="rbpsum", bufs=2, space="PSUM"))
        opsum = ctx.enter_context(tc.tile_pool(name="opsum", bufs=2, space="PSUM"))

        # --- resident weights, loaded on the ACT hwdge queue so they overlap
        # the first xt load (sync queue).  mt is split per k-chunk so the
        # first score matmuls only wait for chunk 0.
        ebias_t = wpool.tile([128, NTT], dt.float32, tag="ebias")
        nc.scalar.dma_start(out=ebias_t[:], in_=ebias[:, :])
        # split the startup-critical loads (first xt rowtile + all M chunks,
        # ~3.5MB) across both hwdge queues in consumption order
        first_xts = []
        for kc in range(ND):
            t = xpool.tile([128, RT], dt.bfloat16, tag=f"xt{kc}")
            nc.sync.dma_start(out=t[:], in_=xt[kc * 128:(kc + 1) * 128, 0:RT])
            first_xts.append(t)
        mt_ts = []
        for kc in range(ND):
            mtc = wpool.tile([128, NT], dt.bfloat16, tag=f"mt{kc}")
            eng = nc.scalar if kc < 6 else nc.sync
            eng.dma_start(out=mtc[:], in_=mt[kc * 128:(kc + 1) * 128, :])
            mt_ts.append(mtc)
        ind_t = wpool.tile([128, NTT * H], dt.float8e4, tag="ind")
        nc.scalar.dma_start(out=ind_t[:], in_=ind[:, :])
        sel_t = wpool.tile([H, NTT * 128], dt.bfloat16, tag="sel")
        nc.scalar.dma_start(out=sel_t[:], in_=sel[:, :])
        vt_t = wpool.tile([128, NTT * D], dt.bfloat16, tag="vt")
        for st in range(NTT):
            nc.scalar.dma_start(out=vt_t[:, st * D:(st + 1) * D],
                                in_=vt[st * 128:(st + 1) * 128, :])

        def pvw_quarter(pat, prt, rc, fillers=None):
            """One 128-row chunk of the previous rowtile's P_norm @ Vt.
            fillers: closures emitting one independent PE op each, interleaved
            every other matmul to overlap softmax bcast with this GEMM."""
            ot = otpool.tile([128, D], dt.bfloat16, tag="ot")
            for oc in range(2):
                op_ = opsum.tile([128, RT], dt.float32, tag="op")
                for st in range(NTT):
                    nc.tensor.matmul(
                        op_[:],
                        lhsT=pat[:, st * RT + rc * 128: st * RT + (rc + 1) * 128],
                        rhs=vt_t[:, st * D + oc * RT: st * D + (oc + 1) * RT],
                        start=(st == 0), stop=(st == NTT - 1),
                    )
                    if fillers and st % 2 == 1:
                        fillers.pop(0)()
                if oc == 0:
                    nc.scalar.copy(ot[:, oc * RT:(oc + 1) * RT], op_[:])
                else:
                    nc.vector.tensor_copy(ot[:, oc * RT:(oc + 1) * RT], op_[:])
                dma_eng = nc.sync if (2 * rc + oc) % 2 == 0 else nc.scalar
                dma_eng.dma_start(
                    out=out[prt * RT + rc * 128: prt * RT + (rc + 1) * 128,
                            oc * RT:(oc + 1) * RT],
                    in_=ot[:, oc * RT:(oc + 1) * RT],
                )

        def load_xt(rt):
            ts = []
            for kc in range(ND):
                t = xpool.tile([128, RT], dt.bfloat16, tag=f"xt{kc}")
                nc.sync.dma_start(
                    out=t[:],
                    in_=xt[kc * 128:(kc + 1) * 128, rt * RT:(rt + 1) * RT])
                ts.append(t)
            return ts

        def score_group(xts, pt_t, pt8_t, st):
            sp = spsum.tile([128, RT], dt.float32, tag="sp")
            for kc in range(ND):
                nc.tensor.matmul(
                    sp[:],
                    lhsT=mt_ts[kc][:, st * 128:(st + 1) * 128],
                    rhs=xts[kc][:],
                    start=(kc == 0), stop=(kc == ND - 1),
                )
            nc.scalar.activation(pt_t[:, st * RT:(st + 1) * RT], sp[:],
                                 mybir.ActivationFunctionType.Exp,
                                 bias=ebias_t[:, st:st + 1])
            # fp8 shadow of P feeds the DoubleRow row-sum matmuls (2x rate)
            nc.scalar.activation(pt8_t[:, st * RT:(st + 1) * RT], sp[:],
                                 mybir.ActivationFunctionType.Exp,
                                 bias=ebias_t[:, st:st + 1])

        def sums_pair(su_h, pt8_t, pr, half, start, stop):
            nc.tensor.matmul(
                su_h[:],
                lhsT=ind_t[:, 2 * pr * H:(2 * pr + 2) * H].rearrange(
                    "p (i h) -> p i h", i=2),
                rhs=pt8_t[:].rearrange("p (t r) -> p t r", t=NTT)[
                    :, 2 * pr:2 * pr + 2, half * 256:(half + 1) * 256],
                start=start, stop=stop,
                perf_mode=mybir.MatmulPerfMode.DoubleRow,
            )

        NPULL = 4  # next-rowtile score groups pulled in to cover 1/s latency
        xts = first_xts
        pt_t = ptpool.tile([128, NTT * RT], dt.bfloat16, tag="pt")
        pt8_t = pt8pool.tile([128, NTT * RT], dt.float8e4, tag="pt8")
        nxt = (xts, pt_t, pt8_t)
        prev = None
        for rt in range(NRT):
            xts, pt_t, pt8_t = nxt
            for st in range(0 if rt == 0 else NPULL, NTT):
                score_group(xts, pt_t, pt8_t, st)

            # --- all-head row sums -> 2x [16, 256] PSUM (fp8 DoubleRow, one
            # bank per half so the two accumulation groups never share a
            # bank).  The last tile-pair is deferred past the pulled score
            # groups so its fp8 P shadow (trailing ACT op) is ready.
            su_h0 = supsum.tile([H, 256], dt.float32, tag="su")
            su_h1 = supsum.tile([H, 256], dt.float32, tag="su")
            su_h = [su_h0, su_h1]
            for half in range(2):
                for pr in range(4):
                    sums_pair(su_h[half], pt8_t, pr, half,
                              start=(pr == 0), stop=False)

            # the next rowtile's first score groups
            if rt + 1 < NRT:
                nxts = load_xt(rt + 1)
                npt = ptpool.tile([128, NTT * RT], dt.bfloat16, tag="pt")
                npt8 = pt8pool.tile([128, NTT * RT], dt.float8e4, tag="pt8")
                for st in range(NPULL):
                    score_group(nxts, npt, npt8, st)
                nxt = (nxts, npt, npt8)

            for half in range(2):
                sums_pair(su_h[half], pt8_t, 4, half, start=False, stop=True)
            # 1/s = exp(-ln s) on ACT (writes bf16 directly); the DVE
            # InstReciprocal measures ~3.3us and sits on the critical path.
            rc_f = rcpool.tile([H, RT], dt.float32, tag="rcf")
            rc_b = rcpool.tile([H, RT], dt.bfloat16, tag="rcb")
            for half in range(2):
                nc.scalar.activation(rc_f[:, half * 256:(half + 1) * 256],
                                     su_h[half][:],
                                     mybir.ActivationFunctionType.Ln)
                nc.scalar.activation(rc_b[:, half * 256:(half + 1) * 256],
                                     rc_f[:, half * 256:(half + 1) * 256],
                                     mybir.ActivationFunctionType.Exp,
                                     scale=-1.0)

            if prev is not None:
                pvw_quarter(prev[1], prev[0], 0)
                pvw_quarter(prev[1], prev[0], 1)

            # --- broadcast 1/s to token partitions, normalize P
            # (interleaved into the PVW GEMM so the single-matmul rb groups
            # never stall the PE on the DVE multiply)
            pn_t = pnpool.tile([128, NTT * RT], dt.bfloat16, tag="pn")

            def mk_bcast(st, pt_ref, pn_ref, rc_ref):
                def emit():
                    rb = rbpsum.tile([128, RT], dt.float32, tag="rb")
                    nc.tensor.matmul(
                        rb[:],
                        lhsT=sel_t[:, st * 128:(st + 1) * 128],
                        rhs=rc_ref[:],
                        start=True, stop=True,
                    )
                    nc.vector.tensor_mul(
                        pn_ref[:, st * RT:(st + 1) * RT],
                        pt_ref[:, st * RT:(st + 1) * RT], rb[:],
                    )
                return emit

            fillers = [mk_bcast(st, pt_t, pn_t, rc_b) for st in range(NTT)]
            if prev is not None:
                pvw_quarter(prev[1], prev[0], 2, fillers)
                pvw_quarter(prev[1], prev[0], 3)
            else:
                for f in fillers:
                    f()
            prev = (rt, pn_t)

        # drain: PVW of the final rowtile
        for rc4 in range(4):
            pvw_quarter(prev[1], prev[0], rc4)

    nc.compile()
    return nc


def _get_program():
    if "p" not in _PROG_CACHE:
        _PROG_CACHE["p"] = _build_program()
    return _PROG_CACHE["p"]


def _prep_inputs(x, te, mask, Wq, Wk, Wv, Wo, Wst):
    """Host-side fp32 weight folding + per-core shard maps."""
    Weff = ((Wst[:D] + Wst[D:]) @ Wo).astype(np.float32)

    # packed-token-axis structure: global row g -> head g//HS, token g%HS
    g = np.arange(NT)
    head_of = g // HS
    tok_of = g % HS
    real = tok_of < TT

    ind_np = np.zeros((128, NTT * H), np.float32)
    sel_np = np.zeros((H, NTT * 128), np.float32)
    for st in range(NTT):
        for p in range(128):
            gg = st * 128 + p
            if real[gg]:
                ind_np[p, st * H + head_of[gg]] = 1.0
                sel_np[head_of[gg], st * 128 + p] = 1.0

    mt_b, vt_b, eb_b = [], [], []
    for b in range(B):
        K = ((te[b] @ Wk).reshape(TT, G, HD) * SCALE).astype(np.float32)
        V = (te[b] @ Wv).reshape(TT, G, HD).astype(np.float32)
        Mp = np.zeros((D, NT), np.float32)
        Vp = np.zeros((NT, D), np.float32)
        for h in range(H):
            gq = h // HPG
            Mp[:, h * HS:h * HS + TT] = Wq[:, h * HD:(h + 1) * HD] @ K[:, gq, :].T
            Vp[h * HS:h * HS + TT] = V[:, gq, :] @ Weff[h * HD:(h + 1) * HD, :]
        mt_b.append(Mp.astype(BF16))
        vt_b.append(Vp.astype(BF16))
        # exp bias: 0 for attended tokens, -30 for masked/pad rows
        eb = np.full(NT, -30.0, np.float32)
        eb[real] = np.where(mask[b][tok_of[real]], 0.0, -30.0)
        eb_b.append(np.ascontiguousarray(eb.reshape(NTT, 128).T))

    E4 = ml_dtypes.float8_e4m3fn if hasattr(ml_dtypes, "float8_e4m3fn") else ml_dtypes.float8_e4m3
    ind_np = ind_np.astype(E4)
    sel_np = sel_np.astype(BF16)

    in_maps = []
    for c in range(NCORES):
        b = c // (NCORES // B)
        fr = (c % (NCORES // B)) * FPC
        xc = x[b, fr:fr + FPC].reshape(ROWS, D).astype(BF16)
        in_maps.append({
            "xt": np.ascontiguousarray(xc.T),
            "mt": mt_b[b],
            "vt": vt_b[b],
            "ind": ind_np,
            "sel": sel_np,
            "ebias": eb_b[b],
        })
    return in_maps


def kernel(x, text_embeddings, padding_mask, use_mqa=0, use_qk_norm=0,
           Wq=None, Wk=None, Wv=None, Wo=None, Wst=None):
    global LAST_RESULTS
    x = np.asarray(x, np.float32)
    te = np.asarray(text_embeddings, np.float32)
    mask = np.asarray(padding_mask).astype(bool)
    Wq = np.asarray(Wq, np.float32)
    Wk = np.asarray(Wk, np.float32)
    Wv = np.asarray(Wv, np.float32)
    Wo = np.asarray(Wo, np.float32)
    Wst = np.asarray(Wst, np.float32)
    assert x.shape == (B, T, HW, D) and te.shape == (B, TT, D)

    in_maps = _prep_inputs(x, te, mask, Wq, Wk, Wv, Wo, Wst)
    nc = _get_program()

    res = run_bass_kernel_spmd(nc, in_maps, list(range(NCORES)),
                               trace=TRACE, **TRACE_KWARGS)
    LAST_RESULTS = res

    outp = np.empty((B, T, HW, D), np.float32)
    for c in range(NCORES):
        b = c // (NCORES // B)
        fr = (c % (NCORES // B)) * FPC
        outp[b, fr:fr + FPC] = res.results[c]["out"].astype(np.float32).reshape(FPC, HW, D)
    return outp
